# revision 85
# baseline (speedup 1.0000x reference)
"""nn_CoupFourGAT — Trainium2 Bass kernel.

Host (numpy): STFT framing + rFFT + map_w projection + 3x3 conv + QKV
projections + gate-fusion MLP + decoder (all tiny, ~100 MFLOP total).
Device (8 NeuronCores, SPMD): the dominant masked-softmax GAT attention:
per (batch, re/im, head) triple computes
  E^T = adj^T * exp(scale * K Q^T);  raw = [E @ V | E @ 1]
with the softmax denominator fused in as an extra ones-column of V.
Data-parallel over batch: core c handles batches [16c, 16c+16).

Design (exp/ACT-saturated pipeline, steady state ~1.03us per 2 triples):
 - MM1 (K Q^T) bf16, tight 247 moving cols per (triple, m-chunk);
   3 triples per 32-partition group (bases {0,32,64}).
 - scores psum tile [128, 4, 256] f32 (2 banks, pool bufs=3) holds
   2 triples x 2 m-chunks (m 0:128 / 119:247, 247 query cols each).
 - one exp ACT per pair ([128, 4x247] strided AP, scale folded in,
   bf16 out) -> one DVE adj-mask mul (bf16 2x mode).  The ACT engine
   (0.833ns/col, no fast mode) is the roofline: 64 x ~1.03us.
 - MM2 Vp-stationary: out^T[l, n] = Vp^T E^T accumulated over the two
   m-chunks; 2 matmuls + 2 ldweights per triple (PE dispatch, not
   compute, limits MM2).  Emitted THREE iterations behind MM1 so the
   in-order PE queue never waits on the DVE mask-mul and never blocks
   the next exp.  Outputs packed 4 triples per PSUM bank ([13, 247] at
   partition bases {0,32} x column halves, pool bufs=2); DVE drains
   the 45 used partitions to SBUF; 32 output DMAs.
 - DMA triggers cost ~0.7us each on the Sync queue and all transfers
   share one ~130GB/s queue, so: q/k ships dense ([12, slots, 512]
   slots of [q 256 | ka 128 | kb 128], 3 partition-base DMAs per
   tile), output DMAs carry only the used partitions, and qk tiles
   2..5 are triggered just-in-time inside the loop so output triggers
   are not queued behind them.
"""
import math
import os
import numpy as np
import ml_dtypes

B, N, L, H = 128, 247, 12, 4
NFFT, HOP, FRAMES = 256, 246, 13
LAM = 0.01
NCORES = 8
BLOC = B // NCORES          # 16 batches per core
NTRI = BLOC * 2 * H         # 128 (batch, re/im, head) triples per core
NBIG = 4                    # output big groups per core
TPB = NTRI // NBIG          # 32 triples per output big group
NSLOT = (NTRI + 2) // 3     # 43 slots of 3 triples (bases {0,32,64})
SPT = 8                     # slots per qk tile
NQKT = (NSLOT + SPT - 1) // SPT  # 6 qk tiles
NU = NTRI // 2              # 64 u-iterations (2 triples each)
SCALE = 1.0 / math.sqrt(L)
BF16 = ml_dtypes.bfloat16

_DEVICE_CACHE = {}
LAST_EXEC_NS = None


def _erf(x):
    try:
        from scipy.special import erf
        return erf(x)
    except Exception:
        return np.vectorize(math.erf, otypes=[np.float64])(x)


def _ln(t, g, b, eps=1e-5):
    m = t.mean(-1, keepdims=True)
    v = ((t - m) ** 2).mean(-1, keepdims=True)
    return (t - m) / np.sqrt(v + eps) * g + b


def _softshrink(t):
    return np.where(t > LAM, t - LAM, np.where(t < -LAM, t + LAM, 0.0))


def _leaky(t):
    return np.where(t >= 0, t, 0.01 * t)


def _front(x, map_w, map_b, conv_w, conv_b):
    """x (B,N,L) -> tr, ti (B,N,L): FFT + map + conv + leaky + residual."""
    Bc = x.shape[0]
    xf = x.reshape(Bc, -1)
    pad = np.pad(xf, ((0, 0), (NFFT // 2, NFFT // 2)), mode='reflect')
    idx = np.arange(FRAMES)[:, None] * HOP + np.arange(NFFT)[None, :]
    frames = pad[:, idx]                               # (B, 13, 256)
    spec = np.fft.rfft(frames.astype(np.float64), axis=-1)  # (B,13,129)
    spec = np.swapaxes(spec, 1, 2)[:, :, :L]           # (B, 129, 12)
    xr = spec.real.reshape(Bc, L, 129) @ map_w.T.astype(np.float64) + map_b
    xi = spec.imag.reshape(Bc, L, 129) @ map_w.T.astype(np.float64) + map_b
    xr = xr.reshape(Bc, N, L).astype(np.float32)
    xi = xi.reshape(Bc, N, L).astype(np.float32)
    vec = np.stack([xr, xi], axis=-1)                  # (B, N, L, 2)
    v2 = vec.reshape(Bc, 2, N, L)
    vp = np.pad(v2, ((0, 0), (0, 0), (1, 1), (1, 1)))
    c = np.zeros_like(v2)
    for o in range(2):
        for i in range(2):
            for ky in range(3):
                for kx in range(3):
                    c[:, o] += conv_w[o, i, ky, kx] * vp[:, i, ky:ky + N, kx:kx + L]
    c = c + conv_b[None, :, None, None]
    c = _leaky(c.reshape(Bc, N, L, 2)) + vec
    return c[..., 0], c[..., 1]


def _pack_host(qt, kt, vp, adj):
    """qt/kt: (NC, NTRI, L, N) f32; vp: (NC, NTRI, N, 13) f32; adj (N, N).

    Returns per-core device arrays:
      qk8 (NC, NQKT, 96, SPT, 512) bf16  [3 triples per slot at partition
            bases {0,32,64}; per slot: q 0:247 | ka 256:384 | kb 384:512]
      vpa/vpb (NC, 128, NTRI, 13) bf16      [m-chunks, overlap zeroed in B]
      adjm (128, 4, 247) bf16               [mask blocks A,B,A,B]
    """
    NC = NCORES

    slot = np.zeros((NC, NQKT * SPT * 3, L, 512), np.float32)
    slot[:, :NTRI, :, :N] = qt
    slot[:, :NTRI, :, 256:384] = kt[:, :, :, :128]
    slot[:, :NTRI, :, 384:512] = kt[:, :, :, 119:]
    src = slot.reshape(NC, NQKT, SPT, 3, L, 512)
    qk8 = np.ascontiguousarray(
        src.transpose(0, 1, 3, 4, 2, 5).astype(BF16))  # (NC,NQKT,3,L,SPT,512)

    vpa = vp[:, :, :128, :]                                  # (NC,NTRI,128,13)
    vpb = vp[:, :, 119:, :].copy()
    vpb[:, :, :9, :] = 0.0                                   # kill overlap
    vpa_h = np.ascontiguousarray(vpa.transpose(0, 2, 1, 3)).astype(BF16)
    vpb_h = np.ascontiguousarray(vpb.transpose(0, 2, 1, 3)).astype(BF16)

    adjT = adj.T.astype(np.float32)                          # adjT[m,n]
    mA = adjT[:128, :]                                       # (128, 247)
    mB = adjT[119:, :]
    adjm_h = np.ascontiguousarray(
        np.stack([mA, mB, mA, mB], axis=1)).astype(BF16)     # (128, 4, 247)
    return qk8, vpa_h, vpb_h, adjm_h


def _unpack_raw(rawC):
    """rawC (NC, 32, 45, 512) f32 -> raw (NC, NTRI, N, 13).

    Triple t = g2*4 + ch*2 + k lives at partitions 32k:32k+13 (l dim)
    x cols 256ch + n of group tile g2."""
    NC = NCORES
    Rk = np.stack([rawC[:, :, 0:13], rawC[:, :, 32:45]], axis=2)
    Rk = Rk.reshape(NC, 32, 2, 13, 2, 256)      # [c, g2, k, l, ch, n]
    T = Rk.transpose(0, 1, 4, 2, 5, 3)          # [c, g2, ch, k, n, l]
    return np.ascontiguousarray(
        T.reshape(NC, NTRI, 256, 13)[:, :, :N, :])


def _device_model_numpy(qk8, vpa_h, vpb_h, adjm_h):
    """Numpy mirror of the device program (per core), on packed arrays."""
    f32 = np.float32
    rawC = np.zeros((32, 45, 512), f32)
    po = None
    adjm = adjm_h.astype(f32)                  # (128, 4, 247)
    for u in range(NU):
        ps = np.zeros((128, 4, 247), f32)
        for hh in range(2):
            t = 2 * u + hh
            sg, j = divmod(t, 3)
            ti, s = divmod(sg, SPT)
            sl = qk8[ti, j, :, s].astype(f32)                # (12, 512)
            q, ka, kb = sl[:, :247], sl[:, 256:384], sl[:, 384:512]
            ps[:, 2 * hh] = ka.T @ q
            ps[:, 2 * hh + 1] = kb.T @ q
        e = np.exp(SCALE * ps).astype(BF16).astype(f32)
        em = (e * adjm).astype(BF16).astype(f32)
        if u % 2 == 0:
            po = np.zeros((128, 512), f32)
        for hh in range(2):
            t = 2 * u + hh
            jj = t % 4
            k, ch = jj % 2, jj // 2
            va = vpa_h[:, t, :].astype(f32)
            vb = vpb_h[:, t, :].astype(f32)
            emA, emB = em[:, 2 * hh], em[:, 2 * hh + 1]
            po[32 * k:32 * k + 13, 256 * ch:256 * ch + 247] = (
                va.T @ emA + vb.T @ emB)
        if u % 2 == 1:
            rawC[u // 2] = po[0:45]
    return rawC


def _prune_redundant_waits(nc):
    """Drop sync waits transitively implied by another wait on the same
    instruction: if wait w2's producing instruction itself waited on
    semaphore w.sem >= w.value, then w2 being satisfied implies w is too
    (sem values are monotonic).  Needed because walrus's HW-decoded PE
    matmul struct has a single sync-wait slot, and the tile scheduler
    emits (ACT, PE) wait pairs on PSUM-reuse matmuls where the PE wait
    is implied by the ACT one."""
    from collections import defaultdict
    for fn in nc.m.functions:
        for blk in fn.blocks:
            insts = list(blk.instructions)
            prod = defaultdict(list)
            for inst in insts:
                si = inst.sync_info
                if si is None:
                    continue
                for up in (si.on_update or []):
                    cum = (prod[up.ant_name][-1][0] if prod[up.ant_name]
                           else 0) + (up.update_value or 1)
                    prod[up.ant_name].append((cum, inst))

            def covers(w2, w):
                # True if waiting on w2 guarantees w is already satisfied.
                for cum, p in prod.get(w2.ant_name, []):
                    if cum >= w2.wait_value:
                        psi = p.sync_info
                        for pw in (psi.on_wait or []) if psi else []:
                            if (pw.ant_name == w.ant_name
                                    and pw.wait_value >= w.wait_value):
                                return True
                        return False
                return False

            for inst in insts:
                si = inst.sync_info
                if si is None or not si.on_wait or len(si.on_wait) < 2:
                    continue
                keep = list(si.on_wait)
                changed = True
                while changed and len(keep) > 1:
                    changed = False
                    for w in list(keep):
                        others = [x for x in keep if x is not w]
                        if any(covers(w2, w) for w2 in others):
                            keep.remove(w)
                            changed = True
                            break
                if len(keep) < len(si.on_wait):
                    si.on_wait = keep


def _build_device():
    import concourse.bass as bass
    import concourse.mybir as mybir
    from concourse import bacc, tile

    f32 = mybir.dt.float32
    bf = mybir.dt.bfloat16
    nc = bacc.Bacc("TRN2", target_bir_lowering=False)
    qk_d = nc.declare_dram_parameter("qk", [NQKT, 3, L, SPT, 512], bf, isOutput=False)
    vpa_d = nc.declare_dram_parameter("vpa", [128, NTRI, 13], bf, isOutput=False)
    vpb_d = nc.declare_dram_parameter("vpb", [128, NTRI, 13], bf, isOutput=False)
    adjm_d = nc.declare_dram_parameter("adjm", [128, 4, 247], bf, isOutput=False)
    rawC_d = nc.declare_dram_parameter("rawC", [32, 45, 512], f32, isOutput=True)

    EXP = mybir.ActivationFunctionType.Exp
    with tile.TileContext(nc) as tc:
        with (
            tc.tile_pool(name="const", bufs=1) as cpool,
            tc.tile_pool(name="work", bufs=3) as work,
            tc.tile_pool(name="workm", bufs=4) as workm,
            tc.tile_pool(name="ostage", bufs=4) as opool,
            tc.tile_pool(name="psums", bufs=3, space=bass.MemorySpace.PSUM) as pps,
            tc.tile_pool(name="psumo", bufs=2, space=bass.MemorySpace.PSUM) as ppo,
        ):
            # DMA order: tile-0 q/k first (first MM1 blocks on it), then
            # mask (DVE(0)) and Vp (MM2(0)), then the remaining qk tiles.
            # Input DMAs ride two HWDGE queues (SP + ACT; ACT is idle until
            # the first exp) so the transfers overlap: the ~420KB Vp consts
            # would otherwise delay qk tile 1 past its first use (u=12).
            qk_tiles = {}

            def qk_dma(ti, eng, js=(0, 1, 2)):
                if ti not in qk_tiles:
                    qs_ = cpool.tile([96, SPT, 512], bf, tag=f"qk{ti}")
                    qk_tiles[ti] = qs_
                qs_ = qk_tiles[ti]
                for j in js:
                    eng.dma_start(out=qs_[32 * j:32 * j + L, :, :],
                                  in_=qk_d[ti, j])

            adjm_t = cpool.tile([128, 4, 247], bf, tag="adjm")
            vpa_t = cpool.tile([128, NTRI, 13], bf, tag="vpa")
            vpb_t = cpool.tile([128, NTRI, 13], bf, tag="vpb")
            # qk tiles 2..5 are triggered just-in-time inside the loop so
            # the output-DMA triggers are not stuck behind them in the
            # Sync queue (each trigger costs ~0.7us of queue time).
            # Vp ships in halves: MM2(v) only needs triple columns t=2v,
            # so the first half unblocks the early MM2s ~2us sooner.
            qk_dma(0, nc.sync)
            nc.sync.dma_start(out=adjm_t[:], in_=adjm_d[:, :, :])
            nc.sync.dma_start(out=vpa_t[:, 0:64, :], in_=vpa_d[:, 0:64, :])
            nc.sync.dma_start(out=vpb_t[:, 0:64, :], in_=vpb_d[:, 0:64, :])
            qk_dma(1, nc.sync)
            nc.sync.dma_start(out=vpa_t[:, 64:NTRI, :],
                              in_=vpa_d[:, 64:NTRI, :])
            nc.sync.dma_start(out=vpb_t[:, 64:NTRI, :],
                              in_=vpb_d[:, 64:NTRI, :])

            ems = {}
            po = None
            for u in range(NU + 3):
                # just-in-time qk tile loads: triggered ~10 iterations
                # before first use so output-DMA triggers are not queued
                # behind them at the head.
                if u >= 14 and (u - 14) % 12 == 0 and (u - 14) // 12 + 2 < NQKT:
                    qk_dma((u - 14) // 12 + 2, nc.sync)
                if u < NU:
                    ps = pps.tile([128, 4, 256], f32, tag="ps")
                    for hh in range(2):
                        t = 2 * u + hh
                        sg, j = divmod(t, 3)
                        ti, s = divmod(sg, SPT)
                        base = qk_tiles[ti]
                        qs = base[32 * j:32 * j + L, s, 0:247]
                        ka = base[32 * j:32 * j + L, s, 256:384]
                        kb = base[32 * j:32 * j + L, s, 384:512]
                        nc.tensor.matmul(ps[:, 2 * hh, 0:247], ka, qs,
                                         start=True, stop=True)
                        nc.tensor.matmul(ps[:, 2 * hh + 1, 0:247], kb, qs,
                                         start=True, stop=True)
                    e = work.tile([128, 4, 256], bf, tag="e")
                    nc.scalar.activation(e[:, :, 0:247], ps[:, :, 0:247], EXP,
                                         scale=SCALE)
                    em = workm.tile([128, 4, 256], bf, tag="em")
                    nc.vector.tensor_mul(em[:, :, 0:247], e[:, :, 0:247],
                                         adjm_t[:, :, :])
                    ems[u] = em
                v = u - 3
                if v >= 0:
                    em = ems.pop(v)
                    if v % 2 == 0:
                        po = ppo.tile([128, 512], f32, tag="po")
                    for hh in range(2):
                        t = 2 * v + hh
                        jj = t % 4
                        k, ch = jj % 2, jj // 2
                        out_ap = po[32 * k:32 * k + 13,
                                    256 * ch:256 * ch + 247]
                        nc.tensor.matmul(out_ap, vpa_t[:, t, :],
                                         em[:, 2 * hh, 0:247],
                                         start=True, stop=False)
                        nc.tensor.matmul(out_ap, vpb_t[:, t, :],
                                         em[:, 2 * hh + 1, 0:247],
                                         start=False, stop=True)
                    if v % 2 == 1:
                        g2 = v // 2
                        oG = opool.tile([128, 512], f32, tag="oG")
                        nc.vector.tensor_copy(oG[0:45, :], po[0:45, :])
                        nc.sync.dma_start(out=rawC_d[g2], in_=oG[0:45, :])
    _prune_redundant_waits(nc)
    nc.finalize()
    return nc


def _attention_device(qk8, vpa_h, vpb_h, adjm_h):
    global LAST_EXEC_NS
    from concourse.bass_utils import run_bass_kernel_spmd
    if "nc" not in _DEVICE_CACHE:
        _DEVICE_CACHE["nc"] = _build_device()
    nc = _DEVICE_CACHE["nc"]
    in_maps = []
    for c in range(NCORES):
        in_maps.append({"qk": qk8[c],
                        "vpa": vpa_h[c], "vpb": vpb_h[c], "adjm": adjm_h})
    trace = bool(os.environ.get("KERNEL_TRACE"))
    res = run_bass_kernel_spmd(nc, in_maps, list(range(NCORES)), trace=trace)
    if trace:
        LAST_EXEC_NS = res.exec_time_ns
        _DEVICE_CACHE["last_results"] = res
    return np.stack([res.results[c]["rawC"] for c in range(NCORES)])


def kernel(x, prc, adj, Wq, Wk, Wv, ln_g, ln_b, ln2_g, ln2_b, enc_w, enc_b,
           dec_w, dec_b, map_w, map_b, conv_w, conv_b, g1_w, g1_b,
           gln_g, gln_b, g2_w, g2_b):
    x = np.asarray(x, np.float32)
    prc = np.asarray(prc, np.float32)
    tr, ti = _front(x, map_w, map_b, conv_w, conv_b)

    # QKV projections for both re/im streams: t (B,N,L) @ W[h] (L,L)
    ts = np.stack([tr, ti], axis=1)                    # (B, 2, N, L)
    Q = np.einsum('brnl,hlo->brhon', ts, Wq)           # (B,2,H,L,N) = Q^T
    K = np.einsum('brnl,hlo->brhon', ts, Wk)
    V = np.einsum('brnl,hlo->brhno', ts, Wv)           # (B,2,H,N,L)
    ones = np.ones((B, 2, H, N, 1), np.float32)
    Vp = np.concatenate([V, ones], axis=-1)            # (B,2,H,N,L+1)

    qt = Q.reshape(NCORES, NTRI, L, N).astype(np.float32)
    kt = K.reshape(NCORES, NTRI, L, N).astype(np.float32)
    vp = Vp.reshape(NCORES, NTRI, N, L + 1).astype(np.float32)
    qk8, vpa_h, vpb_h, adjm_h = _pack_host(qt, kt, vp, adj)

    if os.environ.get("KERNEL_NUMPY"):
        rawC = np.stack([_device_model_numpy(qk8[c], vpa_h[c], vpb_h[c],
                                             adjm_h) for c in range(NCORES)])
    else:
        try:
            rawC = _attention_device(qk8, vpa_h, vpb_h, adjm_h)
        except Exception as e:
            import traceback
            traceback.print_exc()
            print(f"DEVICE PATH FAILED ({e}); falling back to numpy")
            rawC = np.stack([_device_model_numpy(qk8[c], vpa_h[c], vpb_h[c],
                                                 adjm_h)
                             for c in range(NCORES)])

    raw = _unpack_raw(rawC)                            # (NC, NTRI, N, 13)
    raw = raw.reshape(B, 2, H, N, L + 1)
    out_av = raw[..., :L] / raw[..., L:L + 1]          # (B,2,H,N,L)
    out_ln = _ln(out_av, ln_g, ln_b)                   # post-attention LN

    res = []
    for ri in range(2):
        out = np.transpose(out_ln[:, ri], (1, 0, 2, 3))  # (H,B,N,L)
        hs = np.transpose(out, (1, 0, 2, 3))             # (B,H,N,L)
        nf = np.moveaxis(hs, 2, 0)                       # (N,B,H,L)
        nfr = nf.reshape(N, H, B, L)
        avg = nfr.mean(axis=1)                           # (N,B,L)
        mx = nfr.max(axis=1)
        z = np.concatenate([avg, mx], axis=-1) @ g1_w.T + g1_b
        z = _ln(z, gln_g, gln_b)
        z = z * 0.5 * (1.0 + _erf(z / math.sqrt(2.0)))   # exact gelu
        z = 1.0 / (1.0 + np.exp(-(z @ g2_w.T + g2_b)))
        fused = z * avg + (1.0 - z) * mx + nf.mean(axis=2)
        res.append(np.transpose(fused, (1, 0, 2)))       # (B,N,L)

    xr = _softshrink(res[0])
    xi = _softshrink(res[1])
    f = xr * enc_w[0, 0] + xi * enc_w[0, 1] + enc_b[0]
    f = _ln(f, ln2_g, ln2_b) + x
    e = f * enc_w[0, 0] + prc * enc_w[0, 1] + enc_b[0]
    a = _leaky(e)
    out = a @ dec_w.T + dec_b + x
    return out.astype(np.float32)
